# revision 5
# baseline (speedup 1.0000x reference)
"""LIF spike-train kernel for Trainium2 (Bass/Tile), data-parallel over 8 cores.

Reference semantics (T=4, tau=0.5, thresh=1.0), per element:
    mem = 0
    for t in range(4):
        mem = mem*0.5 + x[t]
        s[t] = (mem - 1 >= 0)
        mem = mem - s[t]

x: [T*B, C, H, W] = [256, 128, 32, 32] f32, viewed as [4, 64, 128, 1024].
Batch dim (64) is sharded 8-ways; each core streams [4, 8, 128, 1024],
flattened to [T, 128, F=8192] for unit-stride DMA.

Performance structure (the kernel is DVE-bound, not HBM-bound):
- The whole membrane update is ONE fused custom-DVE op per step:
      u' = 0.5*u - 0.5*(u >= 1) + x'
  registered via the documented dve_ops extension point. This keeps the
  spike feedback internal to the op, so per step the DVE runs a single
  2-src pass instead of three (STT + compare + sub).
- The spike OUTPUT compares are then output-only and are split between
  the DVE (tensor_scalar is_ge -> {0,1} i8) and the otherwise-idle ACT
  engine (Sign(u-1) -> {-1,0,1} i8, decoded on the host as z >= 0).
- Spikes are stored as int8 (4x less HBM write traffic than f32); the
  host casts/decodes back to f32 during unshard.

Bit-exactness vs the fp32 reference:
- In the fused op: 0.5*u is exact, (u>=1)*0.5 is exact, and
  0.5*u - 0.5*s = 0.5*(u - s) is exact because u - s is exactly
  representable (u < 2^24); the final +x' is the single rounding,
  identical to the reference's fl(0.5*v + x').
- For the ACT path: fl(u - 1) is exact by Sterbenz whenever u is in
  [0.5, 2], so the sign of u-1 (and the ==0 case, i.e. u exactly 1.0)
  is always decided correctly; host maps z>=0 -> spike, matching is_ge.
"""

import os
import sys

sys.path.insert(0, "/opt/trn_rl_repo")

import numpy as np

T = 4
B = 64
C = 128
HW = 1024
NCORES = 8
BLOC = B // NCORES  # 8 batch elements per core

F = BLOC * C * HW // 128  # 8192
W = min(int(os.environ.get("LIF_W", "2048")), F)
NCH = F // W
assert F % W == 0

# chunks per t whose output compare runs on the DVE ({0,1} encoding);
# the rest run on ACT (sign encoding). Host decode must match.
K_DVE = int(os.environ.get("LIF_DVE_CMP", "1"))
assert 0 <= K_DVE <= NCH

LAST_EXEC_NS = None
LAST_TRACE = None

_CACHE = {}

_LIF_OP_NAME = "LIF_STEP_U_ANT"


def _register_lif_op():
    """Register the fused LIF membrane-update op with dve_ops (documented
    extension point: append to OPS; the per-NEFF uop table is generated from
    it at compile time). out = (in0*s0 - (in0 >= s1)*imm2) + in1."""
    import concourse.dve_ops as dve_ops

    for o in dve_ops.OPS:
        if o.name == _LIF_OP_NAME:
            return o

    from concourse.dve_spec import C0, C1, C2, Spec, Src0, Src1
    from concourse.dve_spec import _has_src1, lower
    from concourse.dve_uop import DveOpSpec

    body = (Src0 * C0 - (Src0 >= C1) * C2) + Src1

    def ref(in0, in1, s0, s1, imm2):
        u = in0.astype(np.float32)
        return (u * s0 - (u >= s1).astype(np.float32) * imm2) + in1

    spec = Spec(body=body, reference=ref)
    shas = {}
    for ver in ("v3", "v4"):
        shas[ver] = DveOpSpec(
            name=_LIF_OP_NAME,
            opcode=0,  # sha covers only the uop table bytes, not the row
            uops=lower(spec, ver=ver),
            rd1_en=_has_src1(spec),
        ).sha(ver)

    op = dve_ops.DveOp(_LIF_OP_NAME, spec, subdim=False, uops_sha=shas)
    dve_ops.OPS.append(op)
    dve_ops.CUSTOM_DVE_SPECS[op.name] = spec
    dve_ops._SUB_OPCODE_FOR_NAME[op.name] = (
        dve_ops._CUSTOM_DVE_ROW_BASE + len(dve_ops.OPS) - 1
    )
    return op


def _build(bloc=BLOC):
    import concourse.bacc as bacc
    import concourse.mybir as mybir
    from concourse import tile

    lif_op = _register_lif_op()

    f32 = mybir.dt.float32
    i8 = mybir.dt.int8
    is_ge = mybir.AluOpType.is_ge

    nc = bacc.Bacc("TRN2", target_bir_lowering=False, debug=False, num_devices=NCORES)
    x = nc.dram_tensor("x", [T, 128, F], f32, kind="ExternalInput").ap()
    y = nc.dram_tensor("y", [T, 128, F], i8, kind="ExternalOutput").ap()

    xbufs = int(os.environ.get("LIF_XBUFS", "10"))
    ubufs = int(os.environ.get("LIF_UBUFS", "6"))
    sbufs = int(os.environ.get("LIF_SBUFS", "6"))
    store_eng_name = os.environ.get("LIF_STORE_ENG", "scalar")

    with tile.TileContext(nc) as tc:
        with tc.tile_pool(name="p", bufs=xbufs) as pool:
            biasm1 = pool.tile([128, 1], f32, bufs=1)
            nc.gpsimd.memset(biasm1, -1.0)

            us = {}
            for t in range(T):
                xs = {}
                for i in range(NCH):
                    xt = pool.tile([128, W], f32, tag="x")
                    nc.sync.dma_start(out=xt, in_=x[t][:, i * W : (i + 1) * W])
                    xs[i] = xt

                if t == 0:
                    us = xs  # u0 = x0
                else:
                    nus = {}
                    for i in range(NCH):
                        u = pool.tile([128, W], f32, tag="u", bufs=ubufs)
                        nc.vector._custom_dve(
                            lif_op,
                            out=u,
                            in0=us[i],
                            in1=xs[i],
                            s0=0.5,
                            s1=1.0,
                            imm2=0.5,
                        )
                        nus[i] = u
                    us = nus

                for i in range(NCH):
                    st = pool.tile([128, W], i8, tag="s", bufs=sbufs)
                    if i < K_DVE:
                        # {0,1} encoding
                        nc.vector.tensor_scalar(st, us[i], 1.0, None, is_ge)
                    else:
                        # {-1,0,1} sign encoding; host decodes z >= 0
                        nc.scalar.activation(
                            st,
                            us[i],
                            mybir.ActivationFunctionType.Sign,
                            bias=biasm1,
                            scale=1.0,
                        )
                    st_eng = nc.scalar if store_eng_name == "scalar" else nc.sync
                    st_eng.dma_start(out=y[t][:, i * W : (i + 1) * W], in_=st)

    nc.compile()
    return nc


def _get_nc():
    if "nc" not in _CACHE:
        _CACHE["nc"] = _build()
    return _CACHE["nc"]


def kernel(x: np.ndarray) -> np.ndarray:
    global LAST_EXEC_NS, LAST_TRACE
    from concourse.bass_utils import run_bass_kernel_spmd

    x = np.ascontiguousarray(np.asarray(x), dtype=np.float32)
    assert x.shape == (T * B, C, 32, 32), x.shape
    xv = x.reshape(T, B, C, HW)

    in_maps = []
    for m in range(NCORES):
        shard = np.ascontiguousarray(xv[:, m * BLOC : (m + 1) * BLOC]).reshape(
            T, 128, F
        )
        in_maps.append({"x": shard})

    nc = _get_nc()
    trace = os.environ.get("LIF_TRACE") == "1"
    res = run_bass_kernel_spmd(nc, in_maps, core_ids=list(range(NCORES)), trace=trace)
    LAST_EXEC_NS = res.exec_time_ns
    if res.instructions_and_trace is not None:
        LAST_TRACE = res.instructions_and_trace[1]

    split = K_DVE * W
    out = np.empty((T, B, C, HW), dtype=np.float32)
    for m in range(NCORES):
        z = res.results[m]["y"]  # int8 [T, 128, F]
        s = np.empty((T, 128, F), dtype=np.float32)
        s[:, :, :split] = z[:, :, :split]  # DVE chunks: already {0,1}
        s[:, :, split:] = z[:, :, split:] >= 0  # ACT chunks: sign decode
        out[:, m * BLOC : (m + 1) * BLOC] = s.reshape(T, BLOC, C, HW)
    return out.reshape(T * B, C, 32, 32)


def _sim_in_out_shape(bloc):
    return (T, 128, bloc * C * HW // 128)


# revision 11
# speedup vs baseline: 1.0261x; 1.0261x over previous
"""LIF spike-train kernel for Trainium2 (Bass/Tile), data-parallel over 8 cores.

Reference semantics (T=4, tau=0.5, thresh=1.0), per element:
    mem = 0
    for t in range(4):
        mem = mem*0.5 + x[t]
        s[t] = (mem - 1 >= 0)
        mem = mem - s[t]

x: [T*B, C, H, W] = [256, 128, 32, 32] f32, viewed as [4, 64, 128, 1024].
Batch dim (64) is sharded 8-ways; each core streams [4, 8, 128, 1024],
flattened to [T, 128, F=8192] for unit-stride DMA.

Performance structure (the kernel is DVE-bound, not HBM-bound):
- The whole membrane update is ONE fused custom-DVE op per step:
      u' = 0.5*u - 0.5*(u >= 1) + x'
  registered via the documented dve_ops extension point. This keeps the
  spike feedback internal to the op, so per step the DVE runs a single
  2-src pass instead of three (STT + compare + sub).
- The spike OUTPUT compares are then output-only and are split between
  the DVE (tensor_scalar is_ge -> {0,1} i8) and the otherwise-idle ACT
  engine (Sign(u-1) -> {-1,0,1} i8, decoded on the host as z >= 0).
- Spikes are stored as int8 (4x less HBM write traffic than f32); the
  host casts/decodes back to f32 during unshard.

Bit-exactness vs the fp32 reference:
- In the fused op: 0.5*u is exact, (u>=1)*0.5 is exact, and
  0.5*u - 0.5*s = 0.5*(u - s) is exact because u - s is exactly
  representable (u < 2^24); the final +x' is the single rounding,
  identical to the reference's fl(0.5*v + x').
- For the ACT path: fl(u - 1) is exact by Sterbenz whenever u is in
  [0.5, 2], so the sign of u-1 (and the ==0 case, i.e. u exactly 1.0)
  is always decided correctly; host maps z>=0 -> spike, matching is_ge.
"""

import os
import sys

sys.path.insert(0, "/opt/trn_rl_repo")

import numpy as np

T = 4
B = 64
C = 128
HW = 1024
NCORES = 8
BLOC = B // NCORES  # 8 batch elements per core

F = BLOC * C * HW // 128  # 8192
W = min(int(os.environ.get("LIF_W", "2048")), F)
NCH = F // W
assert F % W == 0

# chunks per t whose output compare runs on the DVE ({0,1} encoding);
# the rest run on ACT (sign encoding). Host decode must match.
K_DVE = int(os.environ.get("LIF_DVE_CMP", "1"))
assert 0 <= K_DVE <= NCH

LAST_EXEC_NS = None
LAST_TRACE = None

_CACHE = {}

_LIF_OP_NAME = "LIF_STEP_U_ANT"


def _register_lif_op():
    """Register the fused LIF membrane-update op with dve_ops (documented
    extension point: append to OPS; the per-NEFF uop table is generated from
    it at compile time). out = (in0*s0 - (in0 >= s1)*imm2) + in1."""
    import concourse.dve_ops as dve_ops

    for o in dve_ops.OPS:
        if o.name == _LIF_OP_NAME:
            return o

    from concourse.dve_spec import C0, C1, C2, Spec, Src0, Src1
    from concourse.dve_spec import _has_src1, lower
    from concourse.dve_uop import DveOpSpec

    body = (Src0 * C0 - (Src0 >= C1) * C2) + Src1

    def ref(in0, in1, s0, s1, imm2):
        u = in0.astype(np.float32)
        return (u * s0 - (u >= s1).astype(np.float32) * imm2) + in1

    spec = Spec(body=body, reference=ref)
    shas = {}
    for ver in ("v3", "v4"):
        shas[ver] = DveOpSpec(
            name=_LIF_OP_NAME,
            opcode=0,  # sha covers only the uop table bytes, not the row
            uops=lower(spec, ver=ver),
            rd1_en=_has_src1(spec),
        ).sha(ver)

    op = dve_ops.DveOp(_LIF_OP_NAME, spec, subdim=False, uops_sha=shas)
    dve_ops.OPS.append(op)
    dve_ops.CUSTOM_DVE_SPECS[op.name] = spec
    dve_ops._SUB_OPCODE_FOR_NAME[op.name] = (
        dve_ops._CUSTOM_DVE_ROW_BASE + len(dve_ops.OPS) - 1
    )
    return op


def _build(bloc=BLOC):
    import concourse.bacc as bacc
    import concourse.mybir as mybir
    from concourse import tile

    lif_op = _register_lif_op()

    f32 = mybir.dt.float32
    i8 = mybir.dt.int8
    is_ge = mybir.AluOpType.is_ge

    nc = bacc.Bacc("TRN2", target_bir_lowering=False, debug=False, num_devices=NCORES)
    x = nc.dram_tensor("x", [T, 128, F], f32, kind="ExternalInput").ap()
    y = nc.dram_tensor("y", [T, 128, F], i8, kind="ExternalOutput").ap()

    xbufs = int(os.environ.get("LIF_XBUFS", "10"))
    ubufs = int(os.environ.get("LIF_UBUFS", "6"))
    sbufs = int(os.environ.get("LIF_SBUFS", "6"))
    store_eng_name = os.environ.get("LIF_STORE_ENG", "scalar")

    with tile.TileContext(nc) as tc:
        with tc.tile_pool(name="p", bufs=xbufs) as pool:
            biasm1 = pool.tile([128, 1], f32, bufs=1)
            nc.gpsimd.memset(biasm1, -1.0)

            us = {}
            for t in range(T):
                xs = {}
                for i in range(NCH):
                    xt = pool.tile([128, W], f32, tag="x")
                    nc.sync.dma_start(out=xt, in_=x[t][:, i * W : (i + 1) * W])
                    xs[i] = xt

                if t == 0:
                    us = xs  # u0 = x0
                else:
                    nus = {}
                    for i in range(NCH):
                        u = pool.tile([128, W], f32, tag="u", bufs=ubufs)
                        nc.vector._custom_dve(
                            lif_op,
                            out=u,
                            in0=us[i],
                            in1=xs[i],
                            s0=0.5,
                            s1=1.0,
                            imm2=0.5,
                        )
                        nus[i] = u
                    us = nus

                for i in range(NCH):
                    st = pool.tile([128, W], i8, tag="s", bufs=sbufs)
                    if i < K_DVE:
                        # {0,1} encoding
                        nc.vector.tensor_scalar(st, us[i], 1.0, None, is_ge)
                    else:
                        # {-1,0,1} sign encoding; host decodes z >= 0
                        nc.scalar.activation(
                            st,
                            us[i],
                            mybir.ActivationFunctionType.Sign,
                            bias=biasm1,
                            scale=1.0,
                        )
                    st_eng = nc.scalar if store_eng_name == "scalar" else nc.sync
                    st_eng.dma_start(out=y[t][:, i * W : (i + 1) * W], in_=st)

    nc.compile()
    return nc


def _build_raw(bloc=BLOC):
    """Raw bacc build: hand-rolled semaphores, no Tile framework pre/epilogue.

    The Tile version pays ~8us of startup (semaphore init, barriers) and
    ~8us of tail (per-semaphore drain waits across 5 engines). This build
    uses 4 semaphores total and ends with a single wait + 4 clears.

    Engine split (W=2048, NCH=4, chunk 0 compares on DVE):
      SP    : 16 x loads (Q1),               then_inc(LX, 16) each
      DVE   : 12 fused LIF steps (inc UD) + 4 chunk-0 compares (inc CPD)
      ACT   : 12 Sign compares + all 16 spike stores (Q10), inc(SDs, 16)
    Spike encodings: chunk 0 is_ge -> {0,1}; chunks 1-3 Sign(1-u) (uses the
    pre-registered const-1.0 bias AP; scale=-1) -> spike iff z <= 0.
    """
    import concourse.bacc as bacc
    import concourse.mybir as mybir
    from contextlib import ExitStack

    assert W == 2048 and NCH == 4, "raw build is hardcoded for W=2048"

    lif_op = _register_lif_op()

    f32 = mybir.dt.float32
    i8 = mybir.dt.int8
    is_ge = mybir.AluOpType.is_ge
    Sign = mybir.ActivationFunctionType.Sign

    NX = 12  # x ring slots: only the 4 t=0 slots are ever reused (by t=3)
    NU = 6  # u ring slots

    nc = bacc.Bacc("TRN2", target_bir_lowering=False, debug=False, num_devices=NCORES)
    x = nc.dram_tensor("x", [T, 128, F], f32, kind="ExternalInput").ap()
    y = nc.dram_tensor("y", [T, 128, F], i8, kind="ExternalOutput").ap()

    X = [nc.alloc_sbuf_tensor(f"X{k}", [128, W], f32).ap() for k in range(NX)]
    U = [nc.alloc_sbuf_tensor(f"U{k}", [128, W], f32).ap() for k in range(NU)]
    Sd = [nc.alloc_sbuf_tensor(f"Sd{t}", [128, W], i8).ap() for t in range(T)]
    Sa = {
        c: nc.alloc_sbuf_tensor(f"Sa{c}", [128, W], i8).ap()
        for c in range(4 * T)
        if c % 4 != 0
    }

    with ExitStack() as stack:
        block = stack.enter_context(nc.Block())
        # One DMA-completion semaphore per load-index residue (mod 4): a
        # single shared counter is UNSOUND for "load k fully landed" --
        # each load incs +1 from each of the 16 SDMA engines at its own
        # last descriptor, and engines skew across loads, so a shared
        # count of 16k can be reached while a lagging engine still has
        # unlanded slices of load k. With mod-4 rotation, sem[i] >= 16*n
        # is reachable only when its first n loads are each fully landed.
        LXs = [
            stack.enter_context(nc.semaphore(f"LX{k}")) for k in range(4)
        ]  # x loads landed (x16, rotated)
        UD = stack.enter_context(nc.semaphore("UD"))  # LIF steps retired
        CPD = stack.enter_context(nc.semaphore("CPD"))  # DVE compares retired
        SDs = stack.enter_context(nc.semaphore("SDs"))  # spike stores landed (x16)
        ACD = stack.enter_context(nc.semaphore("ACD"))  # ACT Sign writes flushed

        @block.sync
        def _(sp: object):
            for idx in range(4 * T):
                t, i = idx // 4, idx % 4
                if idx >= NX:
                    # overwrites the t=0 slot i: wait for LIF(1,i) (its last
                    # compute read) and store of chunk (0,i) (its last use).
                    ii = idx - NX
                    sp.wait_ge(UD, ii + 1)
                    sp.wait_ge(SDs, 16 * (ii + 1))
                sp.dma_start(
                    out=X[idx % NX], in_=x[t][:, i * W : (i + 1) * W]
                ).then_inc(LXs[idx % 4], 16)

        @block.vector
        def _(ve: object):
            ve.wait_ge(LXs[0], 16)
            ve.tensor_scalar(Sd[0], X[0], 1.0, None, is_ge)
            ve.drain().then_inc(CPD, 1)
            for t in range(1, T):
                for i in range(4):
                    j = 4 * (t - 1) + i
                    ve.wait_ge(LXs[i], 16 * (t + 1))
                    if j >= NU:
                        # reusing u slot of u_{j-NU}: its last reader is the
                        # ACT Sign of chunk c=j-2, which store c=j-2 implies.
                        ve.wait_ge(SDs, 16 * (j - 1))
                    in0 = X[i] if t == 1 else U[(j - 4) % NU]
                    ve._custom_dve(
                        lif_op,
                        out=U[j % NU],
                        in0=in0,
                        in1=X[(4 * t + i) % NX],
                        s0=0.5,
                        s1=1.0,
                        imm2=0.5,
                    )
                    # inc fused onto the drain: a bare then_inc on the
                    # compute op fires at retire (before SBUF writeback), and
                    # a separate sem_inc can issue past a non-blocking drain.
                    ve.drain().then_inc(UD, 1)
                    if i == 0:
                        ve.tensor_scalar(Sd[t], U[j % NU], 1.0, None, is_ge)
                        ve.drain().then_inc(CPD, 1)

        @block.scalar
        def _(act: object):
            nsign = 0
            for c in range(4 * T):
                t, i = c // 4, c % 4
                if i == 0:
                    act.wait_ge(CPD, t + 1)
                    st = Sd[t]
                else:
                    if t == 0:
                        act.wait_ge(LXs[i], 16)
                        usrc = X[i]
                    else:
                        j = 4 * (t - 1) + i
                        act.wait_ge(UD, j + 1)
                        usrc = U[j % NU]
                    # z = Sign(1 - u); spike iff z <= 0 (decoded on host).
                    # bias=1.0 rides the pre-registered const AP.
                    act.activation(Sa[c], usrc, Sign, bias=1.0, scale=-1.0)
                    # The store below is dispatched by this same engine; wait
                    # for the drain-completion inc so the activation's SBUF
                    # writes land before the SDMA engines read the tile.
                    nsign += 1
                    act.drain().then_inc(ACD, 1)
                    act.wait_ge(ACD, nsign)
                    st = Sa[c]
                act.dma_start(out=y[t][:, i * W : (i + 1) * W], in_=st).then_inc(
                    SDs, 16
                )
            act.wait_ge(SDs, 16 * 4 * T)
            for s in (*LXs, UD, CPD, SDs, ACD):
                act.sem_clear(s)

    nc.compile()
    return nc


def _get_nc():
    if "nc" not in _CACHE:
        if os.environ.get("LIF_TILE") == "1":
            _CACHE["nc"] = _build()
            _CACHE["decode"] = "tile"
        else:
            _CACHE["nc"] = _build_raw()
            _CACHE["decode"] = "raw"
    return _CACHE["nc"]


def kernel(x: np.ndarray) -> np.ndarray:
    global LAST_EXEC_NS, LAST_TRACE
    from concourse.bass_utils import run_bass_kernel_spmd

    x = np.ascontiguousarray(np.asarray(x), dtype=np.float32)
    assert x.shape == (T * B, C, 32, 32), x.shape
    xv = x.reshape(T, B, C, HW)

    in_maps = []
    for m in range(NCORES):
        shard = np.ascontiguousarray(xv[:, m * BLOC : (m + 1) * BLOC]).reshape(
            T, 128, F
        )
        in_maps.append({"x": shard})

    nc = _get_nc()
    trace = os.environ.get("LIF_TRACE") == "1"
    res = run_bass_kernel_spmd(nc, in_maps, core_ids=list(range(NCORES)), trace=trace)
    LAST_EXEC_NS = res.exec_time_ns
    if res.instructions_and_trace is not None:
        LAST_TRACE = res.instructions_and_trace[1]

    raw = _CACHE.get("decode") == "raw"
    split = (1 if raw else K_DVE) * W
    out = np.empty((T, B, C, HW), dtype=np.float32)
    for m in range(NCORES):
        z = res.results[m]["y"]  # int8 [T, 128, F]
        s = np.empty((T, 128, F), dtype=np.float32)
        s[:, :, :split] = z[:, :, :split]  # DVE chunks: already {0,1}
        if raw:
            # ACT chunks: z = Sign(1-u); spike iff z <= 0
            s[:, :, split:] = z[:, :, split:] <= 0
        else:
            # ACT chunks: z = Sign(u-1); spike iff z >= 0
            s[:, :, split:] = z[:, :, split:] >= 0
        out[:, m * BLOC : (m + 1) * BLOC] = s.reshape(T, BLOC, C, HW)
    return out.reshape(T * B, C, 32, 32)


def _sim_in_out_shape(bloc):
    return (T, 128, bloc * C * HW // 128)


# revision 12
# speedup vs baseline: 1.1748x; 1.1449x over previous
"""LIF spike-train kernel for Trainium2 (Bass/Tile), data-parallel over 8 cores.

Reference semantics (T=4, tau=0.5, thresh=1.0), per element:
    mem = 0
    for t in range(4):
        mem = mem*0.5 + x[t]
        s[t] = (mem - 1 >= 0)
        mem = mem - s[t]

x: [T*B, C, H, W] = [256, 128, 32, 32] f32, viewed as [4, 64, 128, 1024].
Batch dim (64) is sharded 8-ways; each core streams [4, 8, 128, 1024],
flattened to [T, 128, F=8192] for unit-stride DMA.

Performance structure (the kernel is DVE-bound, not HBM-bound):
- The whole membrane update is ONE fused custom-DVE op per step:
      u' = 0.5*u - 0.5*(u >= 1) + x'
  registered via the documented dve_ops extension point. This keeps the
  spike feedback internal to the op, so per step the DVE runs a single
  2-src pass instead of three (STT + compare + sub).
- The spike OUTPUT compares are then output-only and are split between
  the DVE (tensor_scalar is_ge -> {0,1} i8) and the otherwise-idle ACT
  engine (Sign(u-1) -> {-1,0,1} i8, decoded on the host as z >= 0).
- Spikes are stored as int8 (4x less HBM write traffic than f32); the
  host casts/decodes back to f32 during unshard.

Bit-exactness vs the fp32 reference:
- In the fused op: 0.5*u is exact, (u>=1)*0.5 is exact, and
  0.5*u - 0.5*s = 0.5*(u - s) is exact because u - s is exactly
  representable (u < 2^24); the final +x' is the single rounding,
  identical to the reference's fl(0.5*v + x').
- For the ACT path: fl(u - 1) is exact by Sterbenz whenever u is in
  [0.5, 2], so the sign of u-1 (and the ==0 case, i.e. u exactly 1.0)
  is always decided correctly; host maps z>=0 -> spike, matching is_ge.
"""

import os
import sys

sys.path.insert(0, "/opt/trn_rl_repo")

import numpy as np

T = 4
B = 64
C = 128
HW = 1024
NCORES = 8
BLOC = B // NCORES  # 8 batch elements per core

F = BLOC * C * HW // 128  # 8192
W = min(int(os.environ.get("LIF_W", "2048")), F)
NCH = F // W
assert F % W == 0

# chunks per t whose output compare runs on the DVE ({0,1} encoding);
# the rest run on ACT (sign encoding). Host decode must match.
K_DVE = int(os.environ.get("LIF_DVE_CMP", "1"))
assert 0 <= K_DVE <= NCH

LAST_EXEC_NS = None
LAST_TRACE = None

_CACHE = {}

_LIF_OP_NAME = "LIF_STEP_U_ANT"


def _register_lif_op():
    """Register the fused LIF membrane-update op with dve_ops (documented
    extension point: append to OPS; the per-NEFF uop table is generated from
    it at compile time). out = (in0*s0 - (in0 >= s1)*imm2) + in1."""
    import concourse.dve_ops as dve_ops

    for o in dve_ops.OPS:
        if o.name == _LIF_OP_NAME:
            return o

    from concourse.dve_spec import C0, C1, C2, Spec, Src0, Src1
    from concourse.dve_spec import _has_src1, lower
    from concourse.dve_uop import DveOpSpec

    body = (Src0 * C0 - (Src0 >= C1) * C2) + Src1

    def ref(in0, in1, s0, s1, imm2):
        u = in0.astype(np.float32)
        return (u * s0 - (u >= s1).astype(np.float32) * imm2) + in1

    spec = Spec(body=body, reference=ref)
    shas = {}
    for ver in ("v3", "v4"):
        shas[ver] = DveOpSpec(
            name=_LIF_OP_NAME,
            opcode=0,  # sha covers only the uop table bytes, not the row
            uops=lower(spec, ver=ver),
            rd1_en=_has_src1(spec),
        ).sha(ver)

    op = dve_ops.DveOp(_LIF_OP_NAME, spec, subdim=False, uops_sha=shas)
    dve_ops.OPS.append(op)
    dve_ops.CUSTOM_DVE_SPECS[op.name] = spec
    dve_ops._SUB_OPCODE_FOR_NAME[op.name] = (
        dve_ops._CUSTOM_DVE_ROW_BASE + len(dve_ops.OPS) - 1
    )
    return op


def _build(bloc=BLOC):
    import concourse.bacc as bacc
    import concourse.mybir as mybir
    from concourse import tile

    lif_op = _register_lif_op()

    f32 = mybir.dt.float32
    i8 = mybir.dt.int8
    is_ge = mybir.AluOpType.is_ge

    nc = bacc.Bacc("TRN2", target_bir_lowering=False, debug=False, num_devices=NCORES)
    x = nc.dram_tensor("x", [T, 128, F], f32, kind="ExternalInput").ap()
    y = nc.dram_tensor("y", [T, 128, F], i8, kind="ExternalOutput").ap()

    xbufs = int(os.environ.get("LIF_XBUFS", "10"))
    ubufs = int(os.environ.get("LIF_UBUFS", "6"))
    sbufs = int(os.environ.get("LIF_SBUFS", "6"))
    store_eng_name = os.environ.get("LIF_STORE_ENG", "scalar")

    with tile.TileContext(nc) as tc:
        with tc.tile_pool(name="p", bufs=xbufs) as pool:
            biasm1 = pool.tile([128, 1], f32, bufs=1)
            nc.gpsimd.memset(biasm1, -1.0)

            us = {}
            for t in range(T):
                xs = {}
                for i in range(NCH):
                    xt = pool.tile([128, W], f32, tag="x")
                    nc.sync.dma_start(out=xt, in_=x[t][:, i * W : (i + 1) * W])
                    xs[i] = xt

                if t == 0:
                    us = xs  # u0 = x0
                else:
                    nus = {}
                    for i in range(NCH):
                        u = pool.tile([128, W], f32, tag="u", bufs=ubufs)
                        nc.vector._custom_dve(
                            lif_op,
                            out=u,
                            in0=us[i],
                            in1=xs[i],
                            s0=0.5,
                            s1=1.0,
                            imm2=0.5,
                        )
                        nus[i] = u
                    us = nus

                for i in range(NCH):
                    st = pool.tile([128, W], i8, tag="s", bufs=sbufs)
                    if i < K_DVE:
                        # {0,1} encoding
                        nc.vector.tensor_scalar(st, us[i], 1.0, None, is_ge)
                    else:
                        # {-1,0,1} sign encoding; host decodes z >= 0
                        nc.scalar.activation(
                            st,
                            us[i],
                            mybir.ActivationFunctionType.Sign,
                            bias=biasm1,
                            scale=1.0,
                        )
                    st_eng = nc.scalar if store_eng_name == "scalar" else nc.sync
                    st_eng.dma_start(out=y[t][:, i * W : (i + 1) * W], in_=st)

    nc.compile()
    return nc


def _build_raw(bloc=BLOC):
    """Raw bacc build: hand-rolled semaphores, no Tile framework pre/epilogue.

    The Tile version pays ~8us of startup (semaphore init, barriers) and
    ~8us of tail (per-semaphore drain waits across 5 engines). This build
    uses 4 semaphores total and ends with a single wait + 4 clears.

    Engine split (W=2048, NCH=4, chunk 0 compares on DVE):
      SP    : 16 x loads (Q1),               then_inc(LX, 16) each
      DVE   : 12 fused LIF steps (inc UD) + 4 chunk-0 compares (inc CPD)
      ACT   : 12 Sign compares + all 16 spike stores (Q10), inc(SDs, 16)
    Spike encodings: chunk 0 is_ge -> {0,1}; chunks 1-3 Sign(1-u) (uses the
    pre-registered const-1.0 bias AP; scale=-1) -> spike iff z <= 0.
    """
    import concourse.bacc as bacc
    import concourse.mybir as mybir
    from contextlib import ExitStack

    assert W == 2048 and NCH == 4, "raw build is hardcoded for W=2048"

    lif_op = _register_lif_op()

    f32 = mybir.dt.float32
    i8 = mybir.dt.int8
    is_ge = mybir.AluOpType.is_ge
    Sign = mybir.ActivationFunctionType.Sign

    NX = 12  # x ring slots: only the 4 t=0 slots are ever reused (by t=3)
    NU = 6  # u ring slots

    nc = bacc.Bacc("TRN2", target_bir_lowering=False, debug=False, num_devices=NCORES)
    x = nc.dram_tensor("x", [T, 128, F], f32, kind="ExternalInput").ap()
    y = nc.dram_tensor("y", [T, 128, F], i8, kind="ExternalOutput").ap()

    X = [nc.alloc_sbuf_tensor(f"X{k}", [128, W], f32).ap() for k in range(NX)]
    U = [nc.alloc_sbuf_tensor(f"U{k}", [128, W], f32).ap() for k in range(NU)]
    Sd = [nc.alloc_sbuf_tensor(f"Sd{t}", [128, W], i8).ap() for t in range(T)]
    Sa = {
        c: nc.alloc_sbuf_tensor(f"Sa{c}", [128, W], i8).ap()
        for c in range(4 * T)
        if c % 4 != 0
    }

    with ExitStack() as stack:
        block = stack.enter_context(nc.Block(no_gpsimd_drain=True))
        # One DMA-completion semaphore per load-index residue (mod 4): a
        # single shared counter is UNSOUND for "load k fully landed" --
        # each load incs +1 from each of the 16 SDMA engines at its own
        # last descriptor, and engines skew across loads, so a shared
        # count of 16k can be reached while a lagging engine still has
        # unlanded slices of load k. With mod-4 rotation, sem[i] >= 16*n
        # is reachable only when its first n loads are each fully landed.
        LXs = [
            stack.enter_context(nc.semaphore(f"LX{k}")) for k in range(4)
        ]  # x loads landed (x16, rotated)
        UD = stack.enter_context(nc.semaphore("UD"))  # LIF steps retired
        CPD = stack.enter_context(nc.semaphore("CPD"))  # DVE compares retired
        SDs = stack.enter_context(nc.semaphore("SDs"))  # spike stores landed (x16)
        ACD = stack.enter_context(nc.semaphore("ACD"))  # ACT Sign writes flushed

        @block.sync
        def _(sp: object):
            for idx in range(4 * T):
                t, i = idx // 4, idx % 4
                if idx >= NX:
                    # overwrites the t=0 slot i: wait for LIF(1,i) (its last
                    # compute read) and store of chunk (0,i) (its last use).
                    ii = idx - NX
                    sp.wait_ge(UD, ii + 1)
                    sp.wait_ge(SDs, 16 * (ii + 1))
                sp.dma_start(
                    out=X[idx % NX], in_=x[t][:, i * W : (i + 1) * W]
                ).then_inc(LXs[idx % 4], 16)

        @block.vector
        def _(ve: object):
            ve.wait_ge(LXs[0], 16)
            ve.tensor_scalar(Sd[0], X[0], 1.0, None, is_ge)
            ve.drain().then_inc(CPD, 1)
            for t in range(1, T):
                for i in range(4):
                    j = 4 * (t - 1) + i
                    ve.wait_ge(LXs[i], 16 * (t + 1))
                    if j >= NU:
                        # reusing u slot of u_{j-NU}: its last reader is the
                        # ACT Sign of chunk c=j-2, which store c=j-2 implies.
                        ve.wait_ge(SDs, 16 * (j - 1))
                    in0 = X[i] if t == 1 else U[(j - 4) % NU]
                    ve._custom_dve(
                        lif_op,
                        out=U[j % NU],
                        in0=in0,
                        in1=X[(4 * t + i) % NX],
                        s0=0.5,
                        s1=1.0,
                        imm2=0.5,
                    )
                    # inc fused onto the drain: a bare then_inc on the
                    # compute op fires at retire (before SBUF writeback), and
                    # a separate sem_inc can issue past a non-blocking drain.
                    ve.drain().then_inc(UD, 1)
                    if i == 0:
                        ve.tensor_scalar(Sd[t], U[j % NU], 1.0, None, is_ge)
                        ve.drain().then_inc(CPD, 1)

        @block.scalar
        def _(act: object):
            nsign = 0
            for c in range(4 * T):
                t, i = c // 4, c % 4
                if i == 0:
                    act.wait_ge(CPD, t + 1)
                    st = Sd[t]
                else:
                    if t == 0:
                        act.wait_ge(LXs[i], 16)
                        usrc = X[i]
                    else:
                        j = 4 * (t - 1) + i
                        act.wait_ge(UD, j + 1)
                        usrc = U[j % NU]
                    # z = Sign(1 - u); spike iff z <= 0 (decoded on host).
                    # bias=1.0 rides the pre-registered const AP.
                    act.activation(Sa[c], usrc, Sign, bias=1.0, scale=-1.0)
                    # The store below is dispatched by this same engine; wait
                    # for the drain-completion inc so the activation's SBUF
                    # writes land before the SDMA engines read the tile.
                    nsign += 1
                    act.drain().then_inc(ACD, 1)
                    act.wait_ge(ACD, nsign)
                    st = Sa[c]
                act.dma_start(out=y[t][:, i * W : (i + 1) * W], in_=st).then_inc(
                    SDs, 16
                )
            act.wait_ge(SDs, 16 * 4 * T)
            for s in (*LXs, UD, CPD, SDs, ACD):
                act.sem_clear(s)

    nc.compile()
    return nc


def _get_nc():
    if "nc" not in _CACHE:
        if os.environ.get("LIF_TILE") == "1":
            _CACHE["nc"] = _build()
            _CACHE["decode"] = "tile"
        else:
            _CACHE["nc"] = _build_raw()
            _CACHE["decode"] = "raw"
    return _CACHE["nc"]


def kernel(x: np.ndarray) -> np.ndarray:
    global LAST_EXEC_NS, LAST_TRACE
    from concourse.bass_utils import run_bass_kernel_spmd

    x = np.ascontiguousarray(np.asarray(x), dtype=np.float32)
    assert x.shape == (T * B, C, 32, 32), x.shape
    xv = x.reshape(T, B, C, HW)

    in_maps = []
    for m in range(NCORES):
        shard = np.ascontiguousarray(xv[:, m * BLOC : (m + 1) * BLOC]).reshape(
            T, 128, F
        )
        in_maps.append({"x": shard})

    nc = _get_nc()
    trace = os.environ.get("LIF_TRACE") == "1"
    res = run_bass_kernel_spmd(nc, in_maps, core_ids=list(range(NCORES)), trace=trace)
    LAST_EXEC_NS = res.exec_time_ns
    if res.instructions_and_trace is not None:
        LAST_TRACE = res.instructions_and_trace[1]

    raw = _CACHE.get("decode") == "raw"
    split = (1 if raw else K_DVE) * W
    out = np.empty((T, B, C, HW), dtype=np.float32)
    for m in range(NCORES):
        z = res.results[m]["y"]  # int8 [T, 128, F]
        s = np.empty((T, 128, F), dtype=np.float32)
        s[:, :, :split] = z[:, :, :split]  # DVE chunks: already {0,1}
        if raw:
            # ACT chunks: z = Sign(1-u); spike iff z <= 0
            s[:, :, split:] = z[:, :, split:] <= 0
        else:
            # ACT chunks: z = Sign(u-1); spike iff z >= 0
            s[:, :, split:] = z[:, :, split:] >= 0
        out[:, m * BLOC : (m + 1) * BLOC] = s.reshape(T, BLOC, C, HW)
    return out.reshape(T * B, C, 32, 32)


def _sim_in_out_shape(bloc):
    return (T, 128, bloc * C * HW // 128)


# revision 13
# speedup vs baseline: 1.1784x; 1.0031x over previous
"""LIF spike-train kernel for Trainium2 (Bass/Tile), data-parallel over 8 cores.

Reference semantics (T=4, tau=0.5, thresh=1.0), per element:
    mem = 0
    for t in range(4):
        mem = mem*0.5 + x[t]
        s[t] = (mem - 1 >= 0)
        mem = mem - s[t]

x: [T*B, C, H, W] = [256, 128, 32, 32] f32, viewed as [4, 64, 128, 1024].
Batch dim (64) is sharded 8-ways; each core streams [4, 8, 128, 1024],
flattened to [T, 128, F=8192] for unit-stride DMA.

Performance structure (the kernel is DVE-bound, not HBM-bound):
- The whole membrane update is ONE fused custom-DVE op per step:
      u' = 0.5*u - 0.5*(u >= 1) + x'
  registered via the documented dve_ops extension point. This keeps the
  spike feedback internal to the op, so per step the DVE runs a single
  2-src pass instead of three (STT + compare + sub).
- The spike OUTPUT compares are then output-only and are split between
  the DVE (tensor_scalar is_ge -> {0,1} i8) and the otherwise-idle ACT
  engine (Sign(u-1) -> {-1,0,1} i8, decoded on the host as z >= 0).
- Spikes are stored as int8 (4x less HBM write traffic than f32); the
  host casts/decodes back to f32 during unshard.

Bit-exactness vs the fp32 reference:
- In the fused op: 0.5*u is exact, (u>=1)*0.5 is exact, and
  0.5*u - 0.5*s = 0.5*(u - s) is exact because u - s is exactly
  representable (u < 2^24); the final +x' is the single rounding,
  identical to the reference's fl(0.5*v + x').
- For the ACT path: fl(u - 1) is exact by Sterbenz whenever u is in
  [0.5, 2], so the sign of u-1 (and the ==0 case, i.e. u exactly 1.0)
  is always decided correctly; host maps z>=0 -> spike, matching is_ge.
"""

import os
import sys

sys.path.insert(0, "/opt/trn_rl_repo")

import numpy as np

T = 4
B = 64
C = 128
HW = 1024
NCORES = 8
BLOC = B // NCORES  # 8 batch elements per core

F = BLOC * C * HW // 128  # 8192
W = min(int(os.environ.get("LIF_W", "2048")), F)
NCH = F // W
assert F % W == 0

# chunks per t whose output compare runs on the DVE ({0,1} encoding);
# the rest run on ACT (sign encoding). Host decode must match.
K_DVE = int(os.environ.get("LIF_DVE_CMP", "1"))
assert 0 <= K_DVE <= NCH

LAST_EXEC_NS = None
LAST_TRACE = None

_CACHE = {}

_LIF_OP_NAME = "LIF_STEP_U_ANT"


def _register_lif_op():
    """Register the fused LIF membrane-update op with dve_ops (documented
    extension point: append to OPS; the per-NEFF uop table is generated from
    it at compile time). out = (in0*s0 - (in0 >= s1)*imm2) + in1."""
    import concourse.dve_ops as dve_ops

    for o in dve_ops.OPS:
        if o.name == _LIF_OP_NAME:
            return o

    from concourse.dve_spec import C0, C1, C2, Spec, Src0, Src1
    from concourse.dve_spec import _has_src1, lower
    from concourse.dve_uop import DveOpSpec

    body = (Src0 * C0 - (Src0 >= C1) * C2) + Src1

    def ref(in0, in1, s0, s1, imm2):
        u = in0.astype(np.float32)
        return (u * s0 - (u >= s1).astype(np.float32) * imm2) + in1

    spec = Spec(body=body, reference=ref)
    shas = {}
    for ver in ("v3", "v4"):
        shas[ver] = DveOpSpec(
            name=_LIF_OP_NAME,
            opcode=0,  # sha covers only the uop table bytes, not the row
            uops=lower(spec, ver=ver),
            rd1_en=_has_src1(spec),
        ).sha(ver)

    op = dve_ops.DveOp(_LIF_OP_NAME, spec, subdim=False, uops_sha=shas)
    dve_ops.OPS.append(op)
    dve_ops.CUSTOM_DVE_SPECS[op.name] = spec
    dve_ops._SUB_OPCODE_FOR_NAME[op.name] = (
        dve_ops._CUSTOM_DVE_ROW_BASE + len(dve_ops.OPS) - 1
    )
    return op


def _build(bloc=BLOC):
    import concourse.bacc as bacc
    import concourse.mybir as mybir
    from concourse import tile

    lif_op = _register_lif_op()

    f32 = mybir.dt.float32
    i8 = mybir.dt.int8
    is_ge = mybir.AluOpType.is_ge

    nc = bacc.Bacc("TRN2", target_bir_lowering=False, debug=False, num_devices=NCORES)
    x = nc.dram_tensor("x", [T, 128, F], f32, kind="ExternalInput").ap()
    y = nc.dram_tensor("y", [T, 128, F], i8, kind="ExternalOutput").ap()

    xbufs = int(os.environ.get("LIF_XBUFS", "10"))
    ubufs = int(os.environ.get("LIF_UBUFS", "6"))
    sbufs = int(os.environ.get("LIF_SBUFS", "6"))
    store_eng_name = os.environ.get("LIF_STORE_ENG", "scalar")

    with tile.TileContext(nc) as tc:
        with tc.tile_pool(name="p", bufs=xbufs) as pool:
            biasm1 = pool.tile([128, 1], f32, bufs=1)
            nc.gpsimd.memset(biasm1, -1.0)

            us = {}
            for t in range(T):
                xs = {}
                for i in range(NCH):
                    xt = pool.tile([128, W], f32, tag="x")
                    nc.sync.dma_start(out=xt, in_=x[t][:, i * W : (i + 1) * W])
                    xs[i] = xt

                if t == 0:
                    us = xs  # u0 = x0
                else:
                    nus = {}
                    for i in range(NCH):
                        u = pool.tile([128, W], f32, tag="u", bufs=ubufs)
                        nc.vector._custom_dve(
                            lif_op,
                            out=u,
                            in0=us[i],
                            in1=xs[i],
                            s0=0.5,
                            s1=1.0,
                            imm2=0.5,
                        )
                        nus[i] = u
                    us = nus

                for i in range(NCH):
                    st = pool.tile([128, W], i8, tag="s", bufs=sbufs)
                    if i < K_DVE:
                        # {0,1} encoding
                        nc.vector.tensor_scalar(st, us[i], 1.0, None, is_ge)
                    else:
                        # {-1,0,1} sign encoding; host decodes z >= 0
                        nc.scalar.activation(
                            st,
                            us[i],
                            mybir.ActivationFunctionType.Sign,
                            bias=biasm1,
                            scale=1.0,
                        )
                    st_eng = nc.scalar if store_eng_name == "scalar" else nc.sync
                    st_eng.dma_start(out=y[t][:, i * W : (i + 1) * W], in_=st)

    nc.compile()
    return nc


def _build_raw(bloc=BLOC):
    """Raw bacc build: hand-rolled semaphores, no Tile framework pre/epilogue.

    The Tile version pays ~8us of startup (semaphore init, barriers) and
    ~8us of tail (per-semaphore drain waits across 5 engines). This build
    uses 4 semaphores total and ends with a single wait + 4 clears.

    Engine split (W=2048, NCH=4, chunk 0 compares on DVE):
      SP    : 16 x loads (Q1),               then_inc(LX, 16) each
      DVE   : 12 fused LIF steps (inc UD) + 4 chunk-0 compares (inc CPD)
      ACT   : 12 Sign compares + all 16 spike stores (Q10), inc(SDs, 16)
    Spike encodings: chunk 0 is_ge -> {0,1}; chunks 1-3 Sign(1-u) (uses the
    pre-registered const-1.0 bias AP; scale=-1) -> spike iff z <= 0.
    """
    import concourse.bacc as bacc
    import concourse.mybir as mybir
    from contextlib import ExitStack

    assert W == 2048 and NCH == 4, "raw build is hardcoded for W=2048"

    lif_op = _register_lif_op()

    f32 = mybir.dt.float32
    i8 = mybir.dt.int8
    is_ge = mybir.AluOpType.is_ge
    Sign = mybir.ActivationFunctionType.Sign

    NX = 12  # x ring slots: only the 4 t=0 slots are ever reused (by t=3)
    NU = 6  # u ring slots

    nc = bacc.Bacc(
        "TRN2",
        target_bir_lowering=False,
        debug=False,
        num_devices=NCORES,
        enable_asserts=False,
        enable_partition_id=False,
        monotonic_sem_count=0,
    )
    x = nc.dram_tensor("x", [T, 128, F], f32, kind="ExternalInput").ap()
    y = nc.dram_tensor("y", [T, 128, F], i8, kind="ExternalOutput").ap()

    X = [nc.alloc_sbuf_tensor(f"X{k}", [128, W], f32).ap() for k in range(NX)]
    U = [nc.alloc_sbuf_tensor(f"U{k}", [128, W], f32).ap() for k in range(NU)]
    Sd = [nc.alloc_sbuf_tensor(f"Sd{t}", [128, W], i8).ap() for t in range(T)]
    Sa = {
        c: nc.alloc_sbuf_tensor(f"Sa{c}", [128, W], i8).ap()
        for c in range(4 * T)
        if c % 4 != 0
    }

    with ExitStack() as stack:
        block = stack.enter_context(nc.Block(no_gpsimd_drain=True))
        # One DMA-completion semaphore per load-index residue (mod 4): a
        # single shared counter is UNSOUND for "load k fully landed" --
        # each load incs +1 from each of the 16 SDMA engines at its own
        # last descriptor, and engines skew across loads, so a shared
        # count of 16k can be reached while a lagging engine still has
        # unlanded slices of load k. With mod-4 rotation, sem[i] >= 16*n
        # is reachable only when its first n loads are each fully landed.
        LXs = [
            stack.enter_context(nc.semaphore(f"LX{k}")) for k in range(4)
        ]  # x loads landed (x16, rotated)
        UD = stack.enter_context(nc.semaphore("UD"))  # LIF steps retired
        CPD = stack.enter_context(nc.semaphore("CPD"))  # DVE compares retired
        SDs = stack.enter_context(nc.semaphore("SDs"))  # spike stores landed (x16)
        ACD = stack.enter_context(nc.semaphore("ACD"))  # ACT Sign writes flushed

        @block.sync
        def _(sp: object):
            for idx in range(4 * T):
                t, i = idx // 4, idx % 4
                if idx >= NX:
                    # overwrites the t=0 slot i: wait for LIF(1,i) (its last
                    # compute read) and store of chunk (0,i) (its last use).
                    ii = idx - NX
                    sp.wait_ge(UD, ii + 1)
                    sp.wait_ge(SDs, 16 * (ii + 1))
                sp.dma_start(
                    out=X[idx % NX], in_=x[t][:, i * W : (i + 1) * W]
                ).then_inc(LXs[idx % 4], 16)

        @block.vector
        def _(ve: object):
            ve.wait_ge(LXs[0], 16)
            ve.tensor_scalar(Sd[0], X[0], 1.0, None, is_ge)
            ve.drain().then_inc(CPD, 1)
            for t in range(1, T):
                for i in range(4):
                    j = 4 * (t - 1) + i
                    ve.wait_ge(LXs[i], 16 * (t + 1))
                    if j >= NU:
                        # reusing u slot of u_{j-NU}: its last reader is the
                        # ACT Sign of chunk c=j-2, which store c=j-2 implies.
                        ve.wait_ge(SDs, 16 * (j - 1))
                    in0 = X[i] if t == 1 else U[(j - 4) % NU]
                    ve._custom_dve(
                        lif_op,
                        out=U[j % NU],
                        in0=in0,
                        in1=X[(4 * t + i) % NX],
                        s0=0.5,
                        s1=1.0,
                        imm2=0.5,
                    )
                    # inc fused onto the drain: a bare then_inc on the
                    # compute op fires at retire (before SBUF writeback), and
                    # a separate sem_inc can issue past a non-blocking drain.
                    ve.drain().then_inc(UD, 1)
                    if i == 0:
                        ve.tensor_scalar(Sd[t], U[j % NU], 1.0, None, is_ge)
                        ve.drain().then_inc(CPD, 1)

        @block.scalar
        def _(act: object):
            nsign = 0
            for c in range(4 * T):
                t, i = c // 4, c % 4
                if i == 0:
                    act.wait_ge(CPD, t + 1)
                    st = Sd[t]
                else:
                    if t == 0:
                        act.wait_ge(LXs[i], 16)
                        usrc = X[i]
                    else:
                        j = 4 * (t - 1) + i
                        act.wait_ge(UD, j + 1)
                        usrc = U[j % NU]
                    # z = Sign(1 - u); spike iff z <= 0 (decoded on host).
                    # bias=1.0 rides the pre-registered const AP.
                    act.activation(Sa[c], usrc, Sign, bias=1.0, scale=-1.0)
                    # The store below is dispatched by this same engine; wait
                    # for the drain-completion inc so the activation's SBUF
                    # writes land before the SDMA engines read the tile.
                    nsign += 1
                    act.drain().then_inc(ACD, 1)
                    act.wait_ge(ACD, nsign)
                    st = Sa[c]
                act.dma_start(out=y[t][:, i * W : (i + 1) * W], in_=st).then_inc(
                    SDs, 16
                )
            act.wait_ge(SDs, 16 * 4 * T)
            for s in (*LXs, UD, CPD, SDs, ACD):
                act.sem_clear(s)

    nc.compile()
    return nc


def _get_nc():
    if "nc" not in _CACHE:
        if os.environ.get("LIF_TILE") == "1":
            _CACHE["nc"] = _build()
            _CACHE["decode"] = "tile"
        else:
            _CACHE["nc"] = _build_raw()
            _CACHE["decode"] = "raw"
    return _CACHE["nc"]


def kernel(x: np.ndarray) -> np.ndarray:
    global LAST_EXEC_NS, LAST_TRACE
    from concourse.bass_utils import run_bass_kernel_spmd

    x = np.ascontiguousarray(np.asarray(x), dtype=np.float32)
    assert x.shape == (T * B, C, 32, 32), x.shape
    xv = x.reshape(T, B, C, HW)

    in_maps = []
    for m in range(NCORES):
        shard = np.ascontiguousarray(xv[:, m * BLOC : (m + 1) * BLOC]).reshape(
            T, 128, F
        )
        in_maps.append({"x": shard})

    nc = _get_nc()
    trace = os.environ.get("LIF_TRACE") == "1"
    res = run_bass_kernel_spmd(nc, in_maps, core_ids=list(range(NCORES)), trace=trace)
    LAST_EXEC_NS = res.exec_time_ns
    if res.instructions_and_trace is not None:
        LAST_TRACE = res.instructions_and_trace[1]

    raw = _CACHE.get("decode") == "raw"
    split = (1 if raw else K_DVE) * W
    out = np.empty((T, B, C, HW), dtype=np.float32)
    for m in range(NCORES):
        z = res.results[m]["y"]  # int8 [T, 128, F]
        s = np.empty((T, 128, F), dtype=np.float32)
        s[:, :, :split] = z[:, :, :split]  # DVE chunks: already {0,1}
        if raw:
            # ACT chunks: z = Sign(1-u); spike iff z <= 0
            s[:, :, split:] = z[:, :, split:] <= 0
        else:
            # ACT chunks: z = Sign(u-1); spike iff z >= 0
            s[:, :, split:] = z[:, :, split:] >= 0
        out[:, m * BLOC : (m + 1) * BLOC] = s.reshape(T, BLOC, C, HW)
    return out.reshape(T * B, C, 32, 32)


def _sim_in_out_shape(bloc):
    return (T, 128, bloc * C * HW // 128)
